# revision 7
# baseline (speedup 1.0000x reference)
"""ConvEmbedding kernel for Trainium2 (Bass/Tile), 8-core data-parallel.

out[b,t,c] = bias[c] + sum_k W[c, x[b,t+k-4], k]   (zero outside [0,T))

Strategy
--------
- Data-parallel over batch: core i handles batch row i (B=8 rows, 8 cores),
  no collectives.
- Host-side layout transform: block table [V+1, K*H] fp16 where
  tab[v, k*H+c] = W[c, v, k] + b[c]/K. Row V (sentinel) = b[c]/K tiled K
  times, so out-of-range taps still contribute exactly b/K and the bias
  comes out of the K-tap sum for free (edges included).
- Indirect DMA (one index per dest partition, full free extent per index)
  gathers one source token's [K, H] block per partition: group j loads
  source tokens 120j-4 .. 120j+123 into partitions 0..127.
- The 9-tap "diagonal" reduction out[q,c] = sum_k G[q+k, k*H+c] is done on
  TensorE: taps 1..8 are accumulating matmuls with shifted-identity
  stationary matrices S_k[p, q] = (p == q+k) into PSUM [120, H] fp32; the
  unshifted tap 0 rides the DVE PSUM->SBUF evacuation as a tensor_add.
- HWDGE stores 120 output tokens per group.
- Measured on TRN2: ~140 us/core, absmax rel err ~2.3e-4 (fp16 table).
"""

import numpy as np

import concourse.bass as bass
import concourse.tile as tile
from concourse import bacc, mybir
from concourse.bass_utils import run_bass_kernel_spmd

B, T, H, V, K = 8, 4096, 512, 32000, 9
PAD = K // 2
P = 128           # SBUF partitions = source tokens per group
Q = P - (K - 1)   # 120 output tokens per group


def _ceil_div(a, b):
    return (a + b - 1) // b


def build_nc(T=T, V=V, gather_bufs=8, out_bufs=4, psum_bufs=4):
    NG = _ceil_div(T, Q)
    nc = bacc.Bacc("TRN2", target_bir_lowering=False, debug=False, num_devices=8)
    f32 = mybir.dt.float32
    f16 = mybir.dt.float16
    i32 = mybir.dt.int32
    tab = nc.dram_tensor("tab", [V + 1, K * H], f16, kind="ExternalInput")
    idx = nc.dram_tensor("idx", [P, NG], i32, kind="ExternalInput")
    shifts = nc.dram_tensor("shifts", [P, K * Q], f16, kind="ExternalInput")
    out = nc.dram_tensor("out", [T, H], f32, kind="ExternalOutput")

    with tile.TileContext(nc) as tc:
        with tc.tile_pool(name="const", bufs=1) as cp, \
             tc.tile_pool(name="gather", bufs=gather_bufs) as gp, \
             tc.tile_pool(name="psum", bufs=psum_bufs, space="PSUM") as pp, \
             tc.tile_pool(name="outp", bufs=out_bufs) as op:
            idx_t = cp.tile([P, NG], i32)
            nc.sync.dma_start(out=idx_t[:], in_=idx[:])
            shifts_t = cp.tile([P, K * Q], f16)
            nc.scalar.dma_start(out=shifts_t[:], in_=shifts[:])
            for j in range(NG):
                nt = min(Q, T - j * Q)
                gt = gp.tile([P, K * H], f16)
                nc.gpsimd.indirect_dma_start(
                    out=gt[:],
                    out_offset=None,
                    in_=tab[:],
                    in_offset=bass.IndirectOffsetOnAxis(
                        ap=idx_t[:, j:j + 1], axis=0
                    ),
                )
                pt = pp.tile([Q, H], f32, space="PSUM")
                for k in range(1, K):
                    nc.tensor.matmul(
                        out=pt[:],
                        lhsT=shifts_t[:, k * Q:(k + 1) * Q],
                        rhs=gt[:, k * H:(k + 1) * H],
                        start=(k == 1),
                        stop=(k == K - 1),
                    )
                ot = op.tile([Q, H], f32)
                # tap k=0 is unshifted: fuse it with the PSUM evacuation
                nc.vector.tensor_add(out=ot[:], in0=gt[:Q, 0:H], in1=pt[:])
                nc.sync.dma_start(out=out[:][j * Q:j * Q + nt, :], in_=ot[:nt, :])
    nc.compile()
    return nc


def prep_table(W, b):
    """[V+1, K*H] fp16: tab[v, k*H+c] = W[c,v,k] + b[c]/K; row V = b/K tiled."""
    Vv = W.shape[1]
    blk = W.transpose(1, 2, 0) + (b / K)[None, None, :]     # [V, K, H] f32
    tab = np.empty((Vv + 1, K * H), np.float16)
    tab[:Vv] = blk.reshape(Vv, K * H).astype(np.float16)
    tab[Vv] = np.tile((b / K).astype(np.float16), K)
    return tab


def prep_idx(x_row, V=V):
    """idx [P, NG] i32: idx[p, j] = x[Q*j + p - PAD], sentinel V out of range."""
    Tx = x_row.shape[0]
    NG = _ceil_div(Tx, Q)
    lo, hi = -PAD, Q * (NG - 1) + P - PAD       # source index range covered
    xext = np.full(hi - lo, V, np.int32)
    xext[-lo:-lo + Tx] = x_row
    p = np.arange(P)[:, None]
    j = np.arange(NG)[None, :]
    return np.ascontiguousarray(xext[(Q * j + p - PAD) - lo])


def prep_shifts():
    """[P, K*Q] fp16: S_k[p, q] = 1 iff p == q + k."""
    s = np.zeros((P, K, Q), np.float16)
    for k in range(K):
        s[np.arange(Q) + k, k, np.arange(Q)] = 1.0
    return np.ascontiguousarray(s.reshape(P, K * Q))


def _run(x, W, b, trace=False, **kw):
    x = np.asarray(x)
    W = np.asarray(W, dtype=np.float32)
    b = np.asarray(b, dtype=np.float32)
    nc = build_nc()
    tab = prep_table(W, b)
    shifts = prep_shifts()
    in_maps = [
        {"tab": tab, "idx": prep_idx(x[i]), "shifts": shifts} for i in range(B)
    ]
    res = run_bass_kernel_spmd(nc, in_maps, core_ids=list(range(B)), trace=trace, **kw)
    out = np.stack([r["out"] for r in res.results], axis=0)
    return out, res


def kernel(x, W, b):
    return _run(x, W, b)[0]


# revision 9
# speedup vs baseline: 1.1899x; 1.1899x over previous
"""ConvEmbedding kernel for Trainium2 (Bass/Tile), 8-core data-parallel.

out[b,t,c] = bias[c] + sum_k W[c, x[b,t+k-4], k]   (zero outside [0,T))

Strategy
--------
- Data-parallel over batch: core i handles batch row i (B=8 rows, 8 cores),
  no collectives.
- Host-side layout transform: block table [V+1, K*H] fp16 where
  tab[v, k*H+c] = W[c, v, k] + b[c]/K. Row V (sentinel) = b[c]/K tiled K
  times, so out-of-range taps still contribute exactly b/K and the bias
  comes out of the K-tap sum for free (edges included).
- Indirect DMA (one index per dest partition, full free extent per index)
  gathers one source token's [K, H] block per partition: group j loads
  source tokens 120j-4 .. 120j+123 into partitions 0..127.
- The 9-tap "diagonal" reduction out[q,c] = sum_k G[q+k, k*H+c] is done on
  TensorE: taps 1..8 are accumulating matmuls with shifted-identity
  stationary matrices S_k[p, q] = (p == q+k) into PSUM [120, H] fp32; the
  unshifted tap 0 rides the DVE PSUM->SBUF evacuation as a tensor_add.
- HWDGE stores 120 output tokens per group.
- Measured on TRN2: ~140 us/core, absmax rel err ~2.3e-4 (fp16 table).
"""

import numpy as np

import concourse.bass as bass
import concourse.tile as tile
from concourse import bacc, mybir
from concourse.bass_utils import run_bass_kernel_spmd

B, T, H, V, K = 8, 4096, 512, 32000, 9
PAD = K // 2
P = 128           # SBUF partitions = source tokens per group
Q = P - (K - 1)   # 120 output tokens per group


def _ceil_div(a, b):
    return (a + b - 1) // b


def build_nc(T=T, V=V, gather_bufs=8, out_bufs=4, psum_bufs=4):
    NG = _ceil_div(T, Q)
    nc = bacc.Bacc("TRN2", target_bir_lowering=False, debug=False, num_devices=8)
    f32 = mybir.dt.float32
    f16 = mybir.dt.float16
    i32 = mybir.dt.int32
    tab = nc.dram_tensor("tab", [V + 1, K * H], f16, kind="ExternalInput")
    idx = nc.dram_tensor("idx", [P, NG], i32, kind="ExternalInput")
    shifts = nc.dram_tensor("shifts", [P, K * Q], f16, kind="ExternalInput")
    out = nc.dram_tensor("out", [T, H], f32, kind="ExternalOutput")

    with tile.TileContext(nc) as tc:
        with tc.tile_pool(name="const", bufs=1) as cp, \
             tc.tile_pool(name="gather", bufs=gather_bufs) as gp, \
             tc.tile_pool(name="psum", bufs=psum_bufs, space="PSUM") as pp, \
             tc.tile_pool(name="outp", bufs=out_bufs) as op:
            idx_t = cp.tile([P, NG], i32)
            nc.sync.dma_start(out=idx_t[:], in_=idx[:])
            shifts_t = cp.tile([P, K * Q], f16)
            nc.scalar.dma_start(out=shifts_t[:], in_=shifts[:])
            for j in range(NG):
                nt = min(Q, T - j * Q)
                # Sources consumed by the nt stored outputs; a partial tail
                # group gathers (and contracts over) only nt+K-1 rows instead
                # of wasting DMA on 100+ pure-sentinel rows. PSUM rows >= nt
                # receive partial sums; they are never stored.
                nsrc = P if nt == Q else nt + K - 1
                nq = min(Q, nsrc)
                gt = gp.tile([nsrc, K * H], f16)
                nc.gpsimd.indirect_dma_start(
                    out=gt[:],
                    out_offset=None,
                    in_=tab[:],
                    in_offset=bass.IndirectOffsetOnAxis(
                        ap=idx_t[:nsrc, j:j + 1], axis=0
                    ),
                )
                pt = pp.tile([Q, H], f32, space="PSUM")
                for k in range(1, K):
                    nc.tensor.matmul(
                        out=pt[:],
                        lhsT=shifts_t[:nsrc, k * Q:(k + 1) * Q],
                        rhs=gt[:, k * H:(k + 1) * H],
                        start=(k == 1),
                        stop=(k == K - 1),
                    )
                ot = op.tile([Q, H], f32)
                # tap k=0 is unshifted: fuse it with the PSUM evacuation
                nc.vector.tensor_add(
                    out=ot[:nq, :], in0=gt[:nq, 0:H], in1=pt[:nq, :]
                )
                nc.sync.dma_start(out=out[:][j * Q:j * Q + nt, :], in_=ot[:nt, :])
    nc.compile()
    return nc


def prep_table(W, b):
    """[V+1, K*H] fp16: tab[v, k*H+c] = W[c,v,k] + b[c]/K; row V = b/K tiled."""
    Vv = W.shape[1]
    blk = W.transpose(1, 2, 0) + (b / K)[None, None, :]     # [V, K, H] f32
    tab = np.empty((Vv + 1, K * H), np.float16)
    tab[:Vv] = blk.reshape(Vv, K * H).astype(np.float16)
    tab[Vv] = np.tile((b / K).astype(np.float16), K)
    return tab


def prep_idx(x_row, V=V):
    """idx [P, NG] i32: idx[p, j] = x[Q*j + p - PAD], sentinel V out of range."""
    Tx = x_row.shape[0]
    NG = _ceil_div(Tx, Q)
    lo, hi = -PAD, Q * (NG - 1) + P - PAD       # source index range covered
    xext = np.full(hi - lo, V, np.int32)
    xext[-lo:-lo + Tx] = x_row
    p = np.arange(P)[:, None]
    j = np.arange(NG)[None, :]
    return np.ascontiguousarray(xext[(Q * j + p - PAD) - lo])


def prep_shifts():
    """[P, K*Q] fp16: S_k[p, q] = 1 iff p == q + k."""
    s = np.zeros((P, K, Q), np.float16)
    for k in range(K):
        s[np.arange(Q) + k, k, np.arange(Q)] = 1.0
    return np.ascontiguousarray(s.reshape(P, K * Q))


def _run(x, W, b, trace=False, **kw):
    x = np.asarray(x)
    W = np.asarray(W, dtype=np.float32)
    b = np.asarray(b, dtype=np.float32)
    nc = build_nc()
    tab = prep_table(W, b)
    shifts = prep_shifts()
    in_maps = [
        {"tab": tab, "idx": prep_idx(x[i]), "shifts": shifts} for i in range(B)
    ]
    res = run_bass_kernel_spmd(nc, in_maps, core_ids=list(range(B)), trace=trace, **kw)
    out = np.stack([r["out"] for r in res.results], axis=0)
    return out, res


def kernel(x, W, b):
    return _run(x, W, b)[0]
